# revision 9
# baseline (speedup 1.0000x reference)
"""Linear-chain CRF forward (log partition) on 8 Trainium2 NeuronCores.

Strategy (data-parallel over batch, 16 rows/core):
  The log-space recursion
      alpha_t[b,to] = feats[b,t,to] + LSE_from(alpha_{t-1}[b,from] + trans[from,to])
  is run in exp space:
      A_t = (A_{t-1} @ W') * E_t,   W' = exp(trans - C),  E_t = exp(feats_t)
  with A kept *transposed* on chip as [to (2x128 partitions), b (16 free)], so
  each step is 4 small matmuls (stationary W' chunks, moving A) whose PSUM
  output is already in the layout the next step consumes -- no transposes.
  Every NR steps a per-row scale r = 1/sum(A) is folded into the next E tile
  (off the critical path) and logged; logZ = log(z) - sum(log r) + n_mm*C.

  alpha_0 is seeded by running the same step with A_{-1} = one-hot(START);
  the final transition to STOP is one more matmul (the STOP column of W').
"""

import numpy as np

import concourse.bacc as bacc
import concourse.bass as bass
import concourse.mybir as mybir
import concourse.tile as tile
from concourse.bass_utils import run_bass_kernel_spmd

F32 = mybir.dt.float32
BF16 = mybir.dt.bfloat16
AF = mybir.ActivationFunctionType

B, T, G = 128, 512, 256
NCORES = 8
BC = B // NCORES          # batch rows per core
START, STOP = G - 2, G - 1
C = 6.0                   # per-matmul constant log-scale folded into W'
NR = 32                   # renorm cadence (steps)
TB = 64                   # feats time-block size
NB = T // TB
RENORM_TS = [t for t in range(T) if (t + 1) % NR == 0 and (t + 1) < T]
NRENORM = len(RENORM_TS)  # 15
N_MM = T + 1              # matmuls that carry the e^-C factor

_CACHE: dict = {}


def _build_program() -> bass.Bass:
    nc = bacc.Bacc("TRN2", target_bir_lowering=False, debug=False,
                   num_devices=NCORES)
    featsT = nc.dram_tensor("featsT", [128, T, 2, BC], F32, kind="ExternalInput")
    trans = nc.dram_tensor("trans", [G, G], F32, kind="ExternalInput")
    logz = nc.dram_tensor("logz", [1, BC], F32, kind="ExternalOutput")

    with tile.TileContext(nc) as tc:
        with (
            tc.tile_pool(name="wpool", bufs=1) as wpool,
            tc.tile_pool(name="stage", bufs=2) as stage_pool,
            tc.tile_pool(name="epool", bufs=3) as e_pool,
            tc.tile_pool(name="apool", bufs=3) as a_pool,
            tc.tile_pool(name="escp", bufs=2) as esc_pool,
            tc.tile_pool(name="misc", bufs=1) as misc,
            tc.tile_pool(name="ps0", bufs=2, space="PSUM") as ps0_pool,
            tc.tile_pool(name="ps1", bufs=2, space="PSUM") as ps1_pool,
            tc.tile_pool(name="pss", bufs=1, space="PSUM") as pss_pool,
        ):
            # ---- weights: W'[from,to] = exp(trans - C), as 2 from-chunk tiles
            biasC = wpool.tile([128, 1], F32, name="biasC")
            nc.vector.memset(biasC[:], -C)
            wk = []
            for k in range(2):
                wt = wpool.tile([128, G], F32, name=f"wt{k}")
                nc.sync.dma_start(wt[:], trans[k * 128:(k + 1) * 128, :])
                wb = wpool.tile([128, G], BF16, name=f"wb{k}")
                nc.scalar.activation(wb[:], wt[:], AF.Exp, bias=biasC[:])
                wk.append(wb)

            ones_col = wpool.tile([128, 1], BF16, name="ones_col")
            nc.vector.memset(ones_col[:], 1.0)
            ones_row = wpool.tile([1, 128], BF16, name="ones_row")
            nc.vector.memset(ones_row[:], 1.0)

            rbuf = misc.tile([1, max(NRENORM, 1) * BC], F32, name="rbuf")

            # ---- E pipeline: DMA feats block, exp on ACT
            eblocks = []
            for blk in range(NB):
                st = stage_pool.tile([128, TB * 2 * BC], F32, name=f"st{blk}",
                                     tag="st")
                src = featsT[:, blk * TB:(blk + 1) * TB, :, :]
                nc.sync.dma_start(st[:], src.rearrange("p t c b -> p (t c b)"))
                eb = e_pool.tile([128, TB * 2 * BC], F32, name=f"eb{blk}",
                                 tag="eb")
                nc.scalar.activation(eb[:], st[:], AF.Exp)
                eblocks.append(eb)

            # ---- A_{-1} = one-hot(START) over [to, b]
            a0p = a_pool.tile([128, BC], BF16, name="a0_init", tag="a0")
            nc.vector.memset(a0p[:], 0.0)
            a1p = a_pool.tile([128, BC], BF16, name="a1_init", tag="a1")
            nc.vector.memset(a1p[:], 0.0)
            # one-hot row via DMA (compute engines need 32-aligned partition base)
            nc.sync.dma_start(a1p[START - 128:START - 127, :],
                              ones_row[0:1, 0:BC])

            # ---- recursion
            esc_pending = None  # scaled E tile for the upcoming step
            ri = 0
            for t in range(T):
                if esc_pending is not None:
                    e0 = esc_pending[:, 0:BC]
                    e1 = esc_pending[:, BC:2 * BC]
                    esc_pending = None
                else:
                    eb = eblocks[t // TB]
                    off = (t % TB) * 2 * BC
                    e0 = eb[:, off:off + BC]
                    e1 = eb[:, off + BC:off + 2 * BC]

                ps0 = ps0_pool.tile([128, BC], F32, name=f"ps0_{t}", tag="p0")
                nc.tensor.matmul(ps0[:], wk[0][:, 0:128], a0p[:],
                                 start=True, stop=False)
                nc.tensor.matmul(ps0[:], wk[1][:, 0:128], a1p[:],
                                 start=False, stop=True)
                a0 = a_pool.tile([128, BC], BF16, name=f"a0_{t}", tag="a0")
                nc.vector.tensor_mul(a0[:], ps0[:], e0)

                ps1 = ps1_pool.tile([128, BC], F32, name=f"ps1_{t}", tag="p1")
                nc.tensor.matmul(ps1[:], wk[0][:, 128:256], a0p[:],
                                 start=True, stop=False)
                nc.tensor.matmul(ps1[:], wk[1][:, 128:256], a1p[:],
                                 start=False, stop=True)
                a1 = a_pool.tile([128, BC], BF16, name=f"a1_{t}", tag="a1")
                nc.vector.tensor_mul(a1[:], ps1[:], e1)

                a0p, a1p = a0, a1

                if t in RENORM_TS:
                    # s[b] = sum_to A_t ; r = 1/s, logged and folded into E_{t+1}
                    s_ps = pss_pool.tile([1, BC], F32, name=f"s_{t}", tag="s")
                    nc.tensor.matmul(s_ps[:], ones_col[:], a0p[:],
                                     start=True, stop=False)
                    nc.tensor.matmul(s_ps[:], ones_col[:], a1p[:],
                                     start=False, stop=True)
                    rsc = esc_pool.tile([1, 2 * BC], BF16, name=f"rsc{t}",
                                        tag="rsc")
                    with nc.allow_low_precision(
                        reason="bf16 scale is intentional; applied value is "
                               "logged exactly and cancelled at the end"
                    ):
                        nc.vector.reciprocal(rsc[:, 0:BC], s_ps[:])
                        nc.vector.reciprocal(rsc[:, BC:2 * BC], s_ps[:])
                    # record the *applied* (bf16-rounded) scale exactly
                    nc.vector.tensor_copy(rbuf[:, ri * BC:(ri + 1) * BC],
                                          rsc[:, 0:BC])
                    ri += 1
                    rb_ps = pss_pool.tile([128, 2 * BC], F32, name=f"rb_{t}",
                                          tag="rb")
                    nc.tensor.matmul(rb_ps[:], ones_row[:], rsc[:],
                                     start=True, stop=True)
                    ebn = eblocks[(t + 1) // TB]
                    offn = ((t + 1) % TB) * 2 * BC
                    esc = esc_pool.tile([128, 2 * BC], F32, name=f"esc{t}",
                                        tag="esc")
                    nc.vector.tensor_mul(esc[:], ebn[:, offn:offn + 2 * BC],
                                         rb_ps[:])
                    esc_pending = esc

            # ---- final: transition to STOP = one more matmul with only the
            # STOP column of W' as stationary (M=1 -> result at partition 0)
            zf = pss_pool.tile([1, BC], F32, name="zf", tag="zf")
            nc.tensor.matmul(zf[:], wk[0][:, STOP:STOP + 1], a0p[:],
                             start=True, stop=False)
            nc.tensor.matmul(zf[:], wk[1][:, STOP:STOP + 1], a1p[:],
                             start=False, stop=True)
            logq = misc.tile([1, BC], F32, name="logq")
            nc.scalar.activation(logq[:], zf[:], AF.Ln)
            rlog = misc.tile([1, NRENORM * BC], F32, name="rlog")
            nc.scalar.activation(rlog[:], rbuf[:], AF.Ln)
            slr = misc.tile([1, BC], F32, name="slr")
            nc.vector.tensor_reduce(
                slr[:],
                rlog[0:1, :].rearrange("p (k b) -> p b k", b=BC),
                axis=mybir.AxisListType.X,
                op=mybir.AluOpType.add,
            )
            lz0 = misc.tile([1, BC], F32, name="lz0")
            nc.vector.tensor_sub(lz0[:], logq[:], slr[:])
            lz1 = misc.tile([1, BC], F32, name="lz1")
            nc.vector.tensor_scalar_add(lz1[:], lz0[:], float(N_MM * C))
            nc.sync.dma_start(logz[:, :], lz1[:])

    nc.compile()
    return nc


def _marshal_inputs(feats: np.ndarray, transitions: np.ndarray):
    """Per-core input dicts. feats -> [to%128, t, to//128, b] fp32."""
    trans = np.ascontiguousarray(transitions, dtype=np.float32)
    in_maps = []
    for c in range(NCORES):
        fc = feats[c * BC:(c + 1) * BC]              # [BC, T, G]
        ft = fc.transpose(2, 1, 0)                   # [G, T, BC]
        ft = ft.reshape(2, 128, T, BC).transpose(1, 2, 0, 3)  # [128,T,2,BC]
        in_maps.append({
            "featsT": np.ascontiguousarray(ft, dtype=np.float32),
            "trans": trans,
        })
    return in_maps


def _get_program() -> bass.Bass:
    if "nc" not in _CACHE:
        _CACHE["nc"] = _build_program()
    return _CACHE["nc"]


def _run(feats, transitions, trace=False, **spmd_kwargs):
    nc = _get_program()
    in_maps = _marshal_inputs(np.asarray(feats), np.asarray(transitions))
    res = run_bass_kernel_spmd(nc, in_maps, list(range(NCORES)),
                               trace=trace, **spmd_kwargs)
    total = np.float64(0.0)
    for r in res.results:
        total += np.asarray(r["logz"], dtype=np.float64).sum()
    return np.float32(total), res


def kernel(feats: np.ndarray, mask: np.ndarray, transitions: np.ndarray) -> np.ndarray:
    assert bool(np.all(mask)), "kernel assumes an all-ones mask"
    out, _ = _run(feats, transitions, trace=False)
    return np.asarray(out, dtype=np.float32)


# revision 13
# speedup vs baseline: 10.6752x; 10.6752x over previous
"""Linear-chain CRF forward (log partition) on 8 Trainium2 NeuronCores.

Strategy (data-parallel over batch, 16 rows/core):
  The log-space recursion
      alpha_t[b,to] = feats[b,t,to] + LSE_from(alpha_{t-1}[b,from] + trans[from,to])
  is run in exp space:
      A_t = (A_{t-1} @ W') * E_t,   W' = exp(trans - C),  E_t = exp(feats_t)
  with A kept *transposed* on chip as [to (2x128 partitions), b (16 free)], so
  each step is 4 small matmuls (stationary W' chunks, moving A) whose PSUM
  output is already in the layout the next step consumes -- no transposes.
  Every NR steps a per-row scale r = 1/sum(A) is folded into the next E tile
  (off the critical path) and logged; logZ = log(z) - sum(log r) + n_mm*C.

  alpha_0 is seeded by running the same step with A_{-1} = one-hot(START);
  the final transition to STOP is one more matmul (the STOP column of W').
"""

import numpy as np

import concourse.bacc as bacc
import concourse.bass as bass
import concourse.mybir as mybir
import concourse.tile as tile
from concourse.bass_utils import run_bass_kernel_spmd

F32 = mybir.dt.float32
BF16 = mybir.dt.bfloat16
AF = mybir.ActivationFunctionType

B, T, G = 128, 512, 256
NCORES = 8
BC = B // NCORES          # batch rows per core
START, STOP = G - 2, G - 1
C = 6.0                   # per-matmul constant log-scale folded into W'
NR = 32                   # renorm cadence (steps)
TB = 64                   # feats time-block size
NB = T // TB
RENORM_TS = [t for t in range(T) if (t + 1) % NR == 0 and (t + 1) < T]
NRENORM = len(RENORM_TS)  # 15
N_MM = T + 1              # matmuls that carry the e^-C factor

_CACHE: dict = {}


def _build_program(repeat: int = 1) -> bass.Bass:
    """repeat>1 re-runs the whole E-pipeline + recursion (timing only)."""
    nc = bacc.Bacc("TRN2", target_bir_lowering=False, debug=False,
                   num_devices=NCORES)
    featsT = nc.dram_tensor("featsT", [128, T, 2, BC], F32, kind="ExternalInput")
    trans = nc.dram_tensor("trans", [G, G], F32, kind="ExternalInput")
    logz = nc.dram_tensor("logz", [1, BC], F32, kind="ExternalOutput")

    with tile.TileContext(nc) as tc:
        with (
            tc.tile_pool(name="wpool", bufs=1) as wpool,
            tc.tile_pool(name="stage", bufs=2) as stage_pool,
            tc.tile_pool(name="epool", bufs=3) as e_pool,
            tc.tile_pool(name="apool", bufs=3) as a_pool,
            tc.tile_pool(name="escp", bufs=2) as esc_pool,
            tc.tile_pool(name="misc", bufs=1) as misc,
            tc.tile_pool(name="ps0", bufs=2, space="PSUM") as ps0_pool,
            tc.tile_pool(name="ps1", bufs=2, space="PSUM") as ps1_pool,
            tc.tile_pool(name="pss", bufs=1, space="PSUM") as pss_pool,
        ):
            # ---- weights: W'[from,to] = exp(trans - C), as 2 from-chunk tiles
            biasC = wpool.tile([128, 1], F32, name="biasC")
            nc.vector.memset(biasC[:], -C)
            wk = []
            for k in range(2):
                wt = wpool.tile([128, G], F32, name=f"wt{k}")
                nc.sync.dma_start(wt[:], trans[k * 128:(k + 1) * 128, :])
                wb = wpool.tile([128, G], BF16, name=f"wb{k}")
                nc.scalar.activation(wb[:], wt[:], AF.Exp, bias=biasC[:])
                wk.append(wb)

            ones_col = wpool.tile([128, 1], BF16, name="ones_col")
            nc.vector.memset(ones_col[:], 1.0)
            ones_row = wpool.tile([1, 128], BF16, name="ones_row")
            nc.vector.memset(ones_row[:], 1.0)

            rbuf = misc.tile([1, max(NRENORM, 1) * BC], F32, name="rbuf")

            def one_pass(rep: int):
                """E-pipeline + full recursion; returns final (a0, a1)."""
                eblocks = []
                for blk in range(NB):
                    st = stage_pool.tile([128, TB * 2 * BC], F32,
                                         name=f"st{rep}_{blk}", tag="st")
                    src = featsT[:, blk * TB:(blk + 1) * TB, :, :]
                    nc.sync.dma_start(st[:],
                                      src.rearrange("p t c b -> p (t c b)"))
                    eb = e_pool.tile([128, TB * 2 * BC], F32,
                                     name=f"eb{rep}_{blk}", tag="eb")
                    nc.scalar.activation(eb[:], st[:], AF.Exp)
                    eblocks.append(eb)

                # A_{-1} = one-hot(START) over [to, b]
                a0p = a_pool.tile([128, BC], BF16, name=f"a0i{rep}", tag="a0")
                nc.vector.memset(a0p[:], 0.0)
                a1p = a_pool.tile([128, BC], BF16, name=f"a1i{rep}", tag="a1")
                nc.vector.memset(a1p[:], 0.0)
                # one-hot row via DMA (engines need 32-aligned partition base)
                nc.sync.dma_start(a1p[START - 128:START - 127, :],
                                  ones_row[0:1, 0:BC])

                esc_pending = None  # scaled E tile for the upcoming step
                ri = 0
                for t in range(T):
                    if esc_pending is not None:
                        e0 = esc_pending[:, 0:BC]
                        e1 = esc_pending[:, BC:2 * BC]
                        esc_pending = None
                    else:
                        eb = eblocks[t // TB]
                        off = (t % TB) * 2 * BC
                        e0 = eb[:, off:off + BC]
                        e1 = eb[:, off + BC:off + 2 * BC]

                    ps0 = ps0_pool.tile([128, BC], F32,
                                        name=f"ps0_{rep}_{t}", tag="p0")
                    nc.tensor.matmul(ps0[:], wk[0][:, 0:128], a0p[:],
                                     start=True, stop=False)
                    nc.tensor.matmul(ps0[:], wk[1][:, 0:128], a1p[:],
                                     start=False, stop=True)
                    a0 = a_pool.tile([128, BC], BF16,
                                     name=f"a0_{rep}_{t}", tag="a0")
                    nc.vector.tensor_mul(a0[:], ps0[:], e0)

                    ps1 = ps1_pool.tile([128, BC], F32,
                                        name=f"ps1_{rep}_{t}", tag="p1")
                    nc.tensor.matmul(ps1[:], wk[0][:, 128:256], a0p[:],
                                     start=True, stop=False)
                    nc.tensor.matmul(ps1[:], wk[1][:, 128:256], a1p[:],
                                     start=False, stop=True)
                    a1 = a_pool.tile([128, BC], BF16,
                                     name=f"a1_{rep}_{t}", tag="a1")
                    nc.vector.tensor_mul(a1[:], ps1[:], e1)

                    a0p, a1p = a0, a1

                    if t in RENORM_TS:
                        # s[b]=sum_to A_t; r=1/s logged, folded into E_{t+1}
                        s_ps = pss_pool.tile([1, BC], F32,
                                             name=f"s_{rep}_{t}", tag="s")
                        nc.tensor.matmul(s_ps[:], ones_col[:], a0p[:],
                                         start=True, stop=False)
                        nc.tensor.matmul(s_ps[:], ones_col[:], a1p[:],
                                         start=False, stop=True)
                        rsc = esc_pool.tile([1, 2 * BC], BF16,
                                            name=f"rsc{rep}_{t}", tag="rsc")
                        with nc.allow_low_precision(
                            reason="bf16 scale is intentional; applied value "
                                   "is logged exactly and cancelled at the end"
                        ):
                            nc.vector.reciprocal(rsc[:, 0:BC], s_ps[:])
                            nc.vector.reciprocal(rsc[:, BC:2 * BC], s_ps[:])
                        # record the *applied* (bf16-rounded) scale exactly
                        nc.vector.tensor_copy(rbuf[:, ri * BC:(ri + 1) * BC],
                                              rsc[:, 0:BC])
                        ri += 1
                        rb_ps = pss_pool.tile([128, 2 * BC], F32,
                                              name=f"rb_{rep}_{t}", tag="rb")
                        nc.tensor.matmul(rb_ps[:], ones_row[:], rsc[:],
                                         start=True, stop=True)
                        ebn = eblocks[(t + 1) // TB]
                        offn = ((t + 1) % TB) * 2 * BC
                        esc = esc_pool.tile([128, 2 * BC], F32,
                                            name=f"esc{rep}_{t}", tag="esc")
                        nc.vector.tensor_mul(esc[:],
                                             ebn[:, offn:offn + 2 * BC],
                                             rb_ps[:])
                        esc_pending = esc
                return a0p, a1p

            for rep in range(repeat):
                a0p, a1p = one_pass(rep)

            # ---- final: transition to STOP = one more matmul with only the
            # STOP column of W' as stationary (M=1 -> result at partition 0)
            zf = pss_pool.tile([1, BC], F32, name="zf", tag="zf")
            nc.tensor.matmul(zf[:], wk[0][:, STOP:STOP + 1], a0p[:],
                             start=True, stop=False)
            nc.tensor.matmul(zf[:], wk[1][:, STOP:STOP + 1], a1p[:],
                             start=False, stop=True)
            logq = misc.tile([1, BC], F32, name="logq")
            nc.scalar.activation(logq[:], zf[:], AF.Ln)
            rlog = misc.tile([1, NRENORM * BC], F32, name="rlog")
            nc.scalar.activation(rlog[:], rbuf[:], AF.Ln)
            slr = misc.tile([1, BC], F32, name="slr")
            nc.vector.tensor_reduce(
                slr[:],
                rlog[0:1, :].rearrange("p (k b) -> p b k", b=BC),
                axis=mybir.AxisListType.X,
                op=mybir.AluOpType.add,
            )
            lz0 = misc.tile([1, BC], F32, name="lz0")
            nc.vector.tensor_sub(lz0[:], logq[:], slr[:])
            lz1 = misc.tile([1, BC], F32, name="lz1")
            nc.vector.tensor_scalar_add(lz1[:], lz0[:], float(N_MM * C))
            nc.sync.dma_start(logz[:, :], lz1[:])

    nc.compile()
    return nc


def _marshal_inputs(feats: np.ndarray, transitions: np.ndarray):
    """Per-core input dicts. feats -> [to%128, t, to//128, b] fp32."""
    trans = np.ascontiguousarray(transitions, dtype=np.float32)
    in_maps = []
    for c in range(NCORES):
        fc = feats[c * BC:(c + 1) * BC]              # [BC, T, G]
        ft = fc.transpose(2, 1, 0)                   # [G, T, BC]
        ft = ft.reshape(2, 128, T, BC).transpose(1, 2, 0, 3)  # [128,T,2,BC]
        in_maps.append({
            "featsT": np.ascontiguousarray(ft, dtype=np.float32),
            "trans": trans,
        })
    return in_maps


def _get_program(repeat: int = 1) -> bass.Bass:
    key = ("nc", repeat)
    if key not in _CACHE:
        _CACHE[key] = _build_program(repeat)
    return _CACHE[key]


def _run(feats, transitions, trace=False, repeat=1, **spmd_kwargs):
    nc = _get_program(repeat)
    in_maps = _marshal_inputs(np.asarray(feats), np.asarray(transitions))
    res = run_bass_kernel_spmd(nc, in_maps, list(range(NCORES)),
                               trace=trace, **spmd_kwargs)
    total = np.float64(0.0)
    for r in res.results:
        total += np.asarray(r["logz"], dtype=np.float64).sum()
    return np.float32(total), res


def kernel(feats: np.ndarray, mask: np.ndarray, transitions: np.ndarray) -> np.ndarray:
    assert bool(np.all(mask)), "kernel assumes an all-ones mask"
    out, _ = _run(feats, transitions, trace=False)
    return np.asarray(out, dtype=np.float32)
